# revision 5
# baseline (speedup 1.0000x reference)
"""Trainium2 Bass kernel v2 for the 2-layer GAT (50k nodes, 800k edges,
8 NeuronCores, SPMD, dst-partitioned edge sharding).

Changes vs v1:
  - Phase A all-bf16: host ships x pre-transposed; W1|WA1 fused into one
    rhs (one matmul pair per window, no on-chip transposes).
  - AllGathers split into two row-halves (windows 0-24 / 25-48); edge
    phases run as two sweeps (src-half 0 then 1) with SBUF aggregation
    accumulators, so each collective overlaps compute.
  - One-hot matrices (both orientations) precomputed on host in fp8e4
    (exact 0/1) and fed straight to the PE as lhsT -> no DVE one-hot
    builds.  Single src table per half (25600 rows < int16 max), no bank
    split; window-merged gather calls with uniform boundary-tile slots.
  - msg multiply split DVE / GPSIMD.
  - Phase D log-softmax batched: per-window exp sums collected, one Ln
    pass + one subtract + one output DMA at the end.
"""
import sys

for _p in ("/opt/trn_rl_repo",):
    if _p not in sys.path:
        sys.path.insert(0, _p)

import numpy as np
import ml_dtypes

from concourse import bass, mybir, bacc
import concourse.tile as tile
from concourse.bass_utils import run_bass_kernel_spmd

P = 128
FP = mybir.dt.float32
I16 = mybir.dt.int16
BF = mybir.dt.bfloat16
F8 = mybir.dt.float8e4
AF = mybir.ActivationFunctionType
OP = mybir.AluOpType
BF_NP = ml_dtypes.bfloat16
F8_NP = ml_dtypes.float8_e4m3


class GATConfig:
    def __init__(self, n_nodes=50000, n_edges=800000, n_cores=8, grp=2,
                 split=1.0):
        self.N = n_nodes
        self.E = n_edges
        self.NC = n_cores
        self.F = 256
        self.H = 8
        self.C1 = 32
        self.C2 = 32
        self.SLAB = self.N // self.NC
        self.WPC = (self.SLAB + P - 1) // P          # 49
        self.SLABP = self.WPC * P                    # 6272
        self.H0W = (self.WPC + 1) // 2               # 25 windows in half 0
        self.H1W = self.WPC - self.H0W               # 24
        self.HW = [self.H0W, self.H1W]
        self.HROWS = [self.H0W * P, self.H1W * P]    # 3200 / 3072
        self.HBASE = [0, self.H0W * P]
        self.ROW1 = 256
        self.ROW2 = 128
        self.GRP = grp
        self.SPLIT = split                            # msg share on DVE
        self.groups = [list(range(g, min(g + grp, self.WPC)))
                       for g in range(0, self.WPC, grp)]


def _wrap16(idx):
    L = len(idx)
    w = np.asarray(idx, np.int16).reshape(L // 16, 16).T  # [16, L//16]
    return np.tile(w, (8, 1))


def preprocess(cfg, edge_index, sort_src=False):
    """Build segment geometry + per-core gather indices and fp8 one-hots.

    Segment = (group of GRP dst windows, src half).  Within a segment the
    per-window edge lists are padded to the max count over cores so window
    boundaries are tile-uniform across cores; boundary tiles get one
    one-hot slot per window present.
    """
    src = np.asarray(edge_index[0], np.int64)
    dst = np.asarray(edge_index[1], np.int64)
    NC, SLAB, SLABP = cfg.NC, cfg.SLAB, cfg.SLABP
    H = cfg.H

    s_loc = src % SLAB
    s_core = src // SLAB
    s_half = (s_loc >= cfg.HROWS[0]).astype(np.int64)
    s_idx = np.where(s_half == 0,
                     s_core * cfg.HROWS[0] + s_loc,
                     s_core * cfg.HROWS[1] + (s_loc - cfg.HBASE[1]))
    d_core = dst // SLAB
    d_loc = dst - d_core * SLAB
    d_win = d_loc // P
    d_slot = d_loc % P

    # edge lists per (core, window, half)
    elists = {}
    for c in range(NC):
        base = np.nonzero(d_core == c)[0]
        for h in range(2):
            sel = base[s_half[base] == h]
            w = d_win[sel]
            for wi in range(cfg.WPC):
                e = sel[w == wi]
                if sort_src:
                    e = e[np.argsort(s_idx[e], kind="stable")]
                elists[(c, wi, h)] = e

    segs = []            # program geometry, shared across cores
    per_core_src = [[] for _ in range(NC)]
    per_core_oht = [[] for _ in range(NC)]
    per_core_ohg = [[] for _ in range(NC)]
    t0 = 0
    s0 = 0
    pad_tot = 0
    edge_tot = 0
    for h in range(2):
        for g in cfg.groups:
            maxc = [max(len(elists[(c, wi, h)]) for c in range(NC))
                    for wi in g]
            offs = np.cumsum([0] + maxc)
            tot = int(offs[-1])
            nt = (tot + P - 1) // P
            npad_tail = nt * P - tot
            # per-tile slots: list of (win) in window order
            slot_list = []          # (slot_i, tile_local, win)
            agg = {wi: [] for wi in g}
            si = 0
            for t in range(nt):
                lo, hi = t * P, (t + 1) * P
                for k, wi in enumerate(g):
                    if offs[k] < hi and offs[k + 1] > lo:
                        slot_list.append((si, t, wi))
                        agg[wi].append(si)
                        si += 1
            ns = si
            segs.append(dict(h=h, g=list(g), nt=nt, ns=ns, t0=t0, s0=s0,
                             offs=[int(o) for o in offs]))
            pad_tot += npad_tail + sum(
                maxc[k] - len(elists[(c, wi, h)])
                for c in range(NC) for k, wi in enumerate(g)) / NC
            edge_tot += tot

            for c in range(NC):
                idx = np.zeros(nt * P, np.int64)
                dlv = np.full(nt * P, -1, np.int64)
                winv = np.full(nt * P, -1, np.int64)
                for k, wi in enumerate(g):
                    e = elists[(c, wi, h)]
                    idx[offs[k]:offs[k] + len(e)] = s_idx[e]
                    dlv[offs[k]:offs[k] + len(e)] = d_slot[e]
                    winv[offs[k]:offs[k] + len(e)] = wi
                per_core_src[c].append(_wrap16(idx))
                oht = np.zeros((P, ns * P), F8_NP)
                ohg = np.zeros((P, ns * P), F8_NP)
                for (si_, t, wi) in slot_list:
                    sl = slice(t * P, (t + 1) * P)
                    dl_t = dlv[sl]
                    wn_t = winv[sl]
                    m = (wn_t == wi) & (dl_t >= 0)
                    e_pos = np.nonzero(m)[0]
                    q = dl_t[e_pos]
                    oht[q, si_ * P + e_pos] = 1.0
                    ohg[e_pos, si_ * P + q] = 1.0
                per_core_oht[c].append(oht)
                per_core_ohg[c].append(ohg)
            t0 += nt
            s0 += ns
    TT, TS = t0, s0
    print(f"preprocess2: TT={TT} tiles TS={TS} slots "
          f"pad={pad_tot/edge_tot*100:.1f}% edges={edge_tot}", flush=True)

    per_core = []
    for c in range(NC):
        per_core.append({
            "srcw": np.concatenate(per_core_src[c], axis=1).astype(np.int16),
            "oht": np.concatenate(per_core_oht[c], axis=1),
            "ohg": np.concatenate(per_core_ohg[c], axis=1),
        })
    return segs, TT, TS, per_core


def build_program(cfg, segs, TT, TS, reps=1, abl=(), nq=4, subnt=4, sp1=False):
    ABL = set(abl)
    NQ = nq
    SUBNT = subnt
    NC, H, C1, C2, F = cfg.NC, cfg.H, cfg.C1, cfg.C2, cfg.F
    ROW1, ROW2, SLABP, WPC = cfg.ROW1, cfg.ROW2, cfg.SLABP, cfg.WPC
    NTMAX = max(s["nt"] for s in segs)
    NSMAX = max(s["ns"] for s in segs)

    nc = bacc.Bacc(None, target_bir_lowering=False, num_devices=NC,
                   num_swdge_queues=NQ)

    xT_d = nc.dram_tensor("xT", [2 * P, SLABP], BF, kind="ExternalInput")
    W1cat_d = nc.dram_tensor("W1cat", [2 * P, ROW1 + H], BF,
                             kind="ExternalInput")
    Rblk_d = nc.dram_tensor("Rblk", [2 * P, P], BF, kind="ExternalInput")
    W2cat_d = nc.dram_tensor("W2cat", [2 * P, C2 + 2], BF,
                             kind="ExternalInput")
    b1_d = nc.dram_tensor("b1rep", [P, H * C1], FP, kind="ExternalInput")
    b2_d = nc.dram_tensor("b2rep", [P, C2], FP, kind="ExternalInput")
    srcw_d = nc.dram_tensor("srcw", [P, TT * 8], I16, kind="ExternalInput")
    oht_d = nc.dram_tensor("oht", [P, TS * P], F8, kind="ExternalInput")
    ohg_d = nc.dram_tensor("ohg", [P, TS * P], F8, kind="ExternalInput")
    out_d = nc.dram_tensor("out", [SLABP, C2], FP, kind="ExternalOutput")

    with tile.TileContext(nc) as tc:
        with (
            tc.tile_pool(name="sbuf", bufs=1) as sb,
            tc.tile_pool(name="psum", bufs=1, space="PSUM") as ps,
            tc.tile_pool(name="dram", bufs=1, space="DRAM") as dp,
        ):
            # ---- constants ----
            W1sb = sb.tile([P, 2, ROW1 + H], BF, tag="W1sb")
            nc.sync.dma_start(out=W1sb[:, 0, :], in_=W1cat_d[0:P, :])
            nc.sync.dma_start(out=W1sb[:, 1, :], in_=W1cat_d[P:2 * P, :])
            Rb = sb.tile([P, 2, P], BF, tag="Rb")
            nc.sync.dma_start(out=Rb[:, 0, :], in_=Rblk_d[0:P, :])
            nc.sync.dma_start(out=Rb[:, 1, :], in_=Rblk_d[P:2 * P, :])
            W2sb = sb.tile([P, 2, C2 + 2], BF, tag="W2sb")
            nc.sync.dma_start(out=W2sb[:, 0, :], in_=W2cat_d[0:P, :])
            nc.sync.dma_start(out=W2sb[:, 1, :], in_=W2cat_d[P:2 * P, :])
            b1sb = sb.tile([P, H * C1], FP, tag="b1sb")
            nc.sync.dma_start(out=b1sb[:], in_=b1_d[:])
            b2sb = sb.tile([P, C2], FP, tag="b2sb")
            nc.sync.dma_start(out=b2sb[:], in_=b2_d[:])
            ident_b = sb.tile([P, P], BF, tag="ident_b")
            from concourse.masks import make_identity
            identf = sb.tile([P, P], FP, tag="identf")
            make_identity(nc, identf[:])
            nc.vector.tensor_copy(out=ident_b[:], in_=identf[:])
            xsT = sb.tile([P, 2, SLABP], BF, tag="xsT")
            nc.sync.dma_start(out=xsT[:, 0, :], in_=xT_d[0:P, :])
            nc.sync.dma_start(out=xsT[:, 1, :], in_=xT_d[P:2 * P, :])
            sidx_all = sb.tile([P, TT * 8], I16, tag="sidx_all")
            nc.sync.dma_start(out=sidx_all[:], in_=srcw_d[:])

            for _rep in range(reps):
                R = f"r{_rep}"
                t1l = [dp.tile([cfg.HROWS[h], ROW1], BF, tag=f"t1l{h}{R}",
                               name=f"t1l{h}{R}") for h in range(2)]
                t1f = [dp.tile([NC * cfg.HROWS[h], ROW1], BF,
                               addr_space="Shared", tag=f"t1f{h}{R}",
                               name=f"t1f{h}{R}") for h in range(2)]
                t2l = [dp.tile([cfg.HROWS[h], ROW2], BF, tag=f"t2l{h}{R}",
                               name=f"t2l{h}{R}") for h in range(2)]
                t2f = [dp.tile([NC * cfg.HROWS[h], ROW2], BF,
                               addr_space="Shared", tag=f"t2f{h}{R}",
                               name=f"t2f{h}{R}") for h in range(2)]

                adwin1 = sb.tile([P, WPC, H], BF, tag="adwin1")
                adwin2 = sb.tile([P, WPC, 1], BF, tag="adwin2")
                acc1 = sb.tile([P, WPC, ROW1 + H], BF, tag="acc1")
                acc2 = sb.tile([P, WPC, C2 + 1], BF, tag="acc2")
                ybuf = sb.tile([P, WPC, C2], FP, tag="ybuf")
                sxp = sb.tile([P, WPC], FP, tag="sxp")

                # ============ Phase A (bf16), per half ============
                for h in range(2):
                    w0 = 0 if h == 0 else cfg.H0W
                    nw = cfg.HW[h]
                    for k in range(nw):
                        wi = w0 + k
                        hps = ps.tile([P, ROW1 + H], FP, tag="big",
                                      space="PSUM", bufs=2)
                        for ch in range(2):
                            nc.tensor.matmul(
                                out=hps[:],
                                lhsT=xsT[:, ch, wi * P:(wi + 1) * P],
                                rhs=W1sb[:, ch, :],
                                start=(ch == 0), stop=(ch == 1))
                        t1 = sb.tile([P, ROW1], BF, tag="t1", bufs=2)
                        nc.scalar.copy(out=t1[:], in_=hps[:, 0:ROW1])
                        nc.scalar.copy(out=adwin1[:, wi, :],
                                       in_=hps[:, ROW1:ROW1 + H])
                        nc.sync.dma_start(
                            out=t1l[h][k * P:(k + 1) * P, :], in_=t1[:])
                    if "coll" in ABL:
                        nc.sync.dma_start(out=t1f[h][0:cfg.HROWS[h], :],
                                          in_=t1l[h][:])
                    else:
                        nc.gpsimd.collective_compute(
                            "AllGather", OP.bypass,
                            replica_groups=[list(range(NC))],
                            ins=[t1l[h][:]], outs=[t1f[h][:]],
                        )

                # ============ Phase B: two sweeps over src halves =======
                qrr = 0
                ag2_done = [False, False]
                for sweep in range(2):
                    for sg in segs:
                        if sg["h"] != sweep:
                            continue
                        nt, ns, t0, s0 = (sg["nt"], sg["ns"], sg["t0"],
                                          sg["s0"])
                        g = sg["g"]
                        sidx = sidx_all[:, t0 * 8:(t0 + nt) * 8]
                        slot_list, agg = _slots(sg)

                        gbuf = sb.tile([P, NTMAX * ROW1], BF, tag="gbuf",
                                       bufs=2)
                        gv = gbuf[:].rearrange("p (t e) -> p t e", e=ROW1)
                        if "gather" in ABL:
                            nc.sync.dma_start(
                                out=gv[:, 0:nt, :],
                                in_=t1f[sweep][0:P * nt, :].rearrange(
                                    "(t p) e -> p t e", p=P))
                        else:
                            for c0 in range(0, nt, SUBNT):
                                cn = min(SUBNT, nt - c0)
                                nc.gpsimd.dma_gather(
                                    out_ap=gv[:, c0:c0 + cn, :],
                                    in_ap=t1f[sweep][:],
                                    idxs_ap=sidx[:, c0 * 8:(c0 + cn) * 8],
                                    num_idxs=cn * P, num_idxs_reg=cn * P,
                                    elem_size=ROW1, single_packet=sp1,
                                    queue_num=qrr % NQ)
                                qrr += 1
                        ohtb = sb.tile([P, NSMAX * P], F8, tag="ohtb",
                                       bufs=2)
                        nc.sync.dma_start(
                            out=ohtb[:, 0:ns * P],
                            in_=oht_d[:, s0 * P:(s0 + ns) * P])
                        ohgb = sb.tile([P, NSMAX * P], F8, tag="ohgb",
                                       bufs=2)
                        nc.sync.dma_start(
                            out=ohgb[:, 0:ns * P],
                            in_=ohg_d[:, s0 * P:(s0 + ns) * P])
                        ohtv = ohtb[:].rearrange("p (s e) -> p s e", e=P)
                        ohgv = ohgb[:].rearrange("p (s e) -> p s e", e=P)

                        # per-edge dst attention via one-hot^T matmuls
                        adg = ps.tile([P, NTMAX * H], FP, tag="adg",
                                      space="PSUM", bufs=2)
                        by_tile = {}
                        for (si, t, wi) in slot_list:
                            by_tile.setdefault(t, []).append((si, wi))
                        for t in range(nt):
                            sl = by_tile[t]
                            for j, (si, wi) in enumerate(sl):
                                nc.tensor.matmul(
                                    out=adg[:, t * H:(t + 1) * H],
                                    lhsT=ohtv[:, si, :],
                                    rhs=adwin1[:, wi, :],
                                    start=(j == 0), stop=(j == len(sl) - 1))

                        # scores
                        e_t = sb.tile([P, NTMAX * H], FP, tag="e_t", bufs=2)
                        nc.vector.tensor_tensor(
                            out=e_t[:, 0:nt * H].rearrange(
                                "p (t h) -> p t h", h=H)[:, :, :, None],
                            in0=gv[:, 0:nt, :].rearrange(
                                "p t (h c) -> p t h c", c=C1)[:, :, :, 0:1],
                            in1=adg[:, 0:nt * H].rearrange(
                                "p (t h) -> p t h", h=H)[:, :, :, None],
                            op=OP.add)
                        ee = sb.tile([P, NTMAX * H], FP, tag="ee", bufs=2)
                        nc.vector.scalar_tensor_tensor(
                            out=ee[:, 0:nt * H], in0=e_t[:, 0:nt * H],
                            scalar=0.2, in1=e_t[:, 0:nt * H],
                            op0=OP.mult, op1=OP.max)
                        eex = sb.tile([P, NTMAX * H], BF, tag="eex", bufs=2)
                        nc.scalar.activation(out=eex[:, 0:nt * H],
                                             in_=ee[:, 0:nt * H], func=AF.Exp)
                        eexv = eex[:].rearrange("p (t h) -> p t h", h=H)

                        # messages [u*eex | eex], split DVE / gpsimd
                        msg = sb.tile([P, NTMAX * (ROW1 + H)], BF, tag="msg",
                                      bufs=2)
                        msgv = msg[:].rearrange("p (t e) -> p t e",
                                                e=ROW1 + H)
                        kd = max(1, min(nt, int(round(nt * cfg.SPLIT))))
                        for eng, lo, hi in ((nc.vector, 0, kd),
                                            (nc.gpsimd, kd, nt)):
                            if lo >= hi:
                                continue
                            eng.tensor_tensor(
                                out=msgv[:, lo:hi, 0:ROW1].rearrange(
                                    "p t (h c) -> p t h c", c=C1),
                                in0=gv[:, lo:hi, :].rearrange(
                                    "p t (h c) -> p t h c", c=C1),
                                in1=eexv[:, lo:hi, :, None].to_broadcast(
                                    [P, hi - lo, H, C1]),
                                op=OP.mult)
                        nc.scalar.copy(out=msgv[:, 0:nt, ROW1:ROW1 + H],
                                       in_=eexv[:, 0:nt, :])

                        # aggregate per window
                        for wi in g:
                            aggp = ps.tile([P, ROW1 + H], FP, tag="big",
                                           space="PSUM", bufs=2)
                            sl = agg[wi]
                            for j, si in enumerate(sl):
                                t = _slot_tile(slot_list, si)
                                nc.tensor.matmul(
                                    out=aggp[:], lhsT=ohgv[:, si, :],
                                    rhs=msgv[:, t, :],
                                    start=(j == 0), stop=(j == len(sl) - 1))
                            if sweep == 0:
                                nc.scalar.copy(out=acc1[:, wi, :],
                                               in_=aggp[:])
                            else:
                                self_tail1(nc, cfg, sb, ps, acc1, aggp, wi,
                                           Rb, ident_b, b1sb, W2sb,
                                           t2l, adwin2)
                        if sweep == 1 and (
                                (not ag2_done[0] and max(g) >= cfg.H0W - 1)
                                or (not ag2_done[1] and max(g) >= WPC - 1)):
                            h = 0 if not ag2_done[0] else 1
                            ag2_done[h] = True
                            if "coll" in ABL:
                                nc.sync.dma_start(
                                    out=t2f[h][0:cfg.HROWS[h], :],
                                    in_=t2l[h][:])
                            else:
                                nc.gpsimd.collective_compute(
                                    "AllGather", OP.bypass,
                                    replica_groups=[list(range(NC))],
                                    ins=[t2l[h][:]], outs=[t2f[h][:]],
                                )

                # ============ Phase D: two sweeps ============
                for sweep in range(2):
                    for sg in segs:
                        if sg["h"] != sweep:
                            continue
                        nt, ns, t0, s0 = (sg["nt"], sg["ns"], sg["t0"],
                                          sg["s0"])
                        g = sg["g"]
                        sidx = sidx_all[:, t0 * 8:(t0 + nt) * 8]
                        slot_list, agg = _slots(sg)
                        g2 = sb.tile([P, NTMAX * ROW2], BF, tag="g2buf",
                                     bufs=2)
                        g2v = g2[:].rearrange("p (t e) -> p t e", e=ROW2)
                        if "gather2" in ABL:
                            nc.sync.dma_start(
                                out=g2v[:, 0:nt, :],
                                in_=t2f[sweep][0:P * nt, :].rearrange(
                                    "(t p) e -> p t e", p=P))
                        else:
                            for c0 in range(0, nt, SUBNT):
                                cn = min(SUBNT, nt - c0)
                                nc.gpsimd.dma_gather(
                                    out_ap=g2v[:, c0:c0 + cn, :],
                                    in_ap=t2f[sweep][:],
                                    idxs_ap=sidx[:, c0 * 8:(c0 + cn) * 8],
                                    num_idxs=cn * P, num_idxs_reg=cn * P,
                                    elem_size=ROW2, single_packet=False,
                                    queue_num=qrr % NQ)
                                qrr += 1
                        ohtb = sb.tile([P, NSMAX * P], F8, tag="ohtb",
                                       bufs=2)
                        nc.sync.dma_start(
                            out=ohtb[:, 0:ns * P],
                            in_=oht_d[:, s0 * P:(s0 + ns) * P])
                        ohgb = sb.tile([P, NSMAX * P], F8, tag="ohgb",
                                       bufs=2)
                        nc.sync.dma_start(
                            out=ohgb[:, 0:ns * P],
                            in_=ohg_d[:, s0 * P:(s0 + ns) * P])
                        ohtv = ohtb[:].rearrange("p (s e) -> p s e", e=P)
                        ohgv = ohgb[:].rearrange("p (s e) -> p s e", e=P)

                        adg = ps.tile([P, NTMAX * H], FP, tag="adg",
                                      space="PSUM", bufs=2)
                        by_tile = {}
                        for (si, t, wi) in slot_list:
                            by_tile.setdefault(t, []).append((si, wi))
                        for t in range(nt):
                            sl = by_tile[t]
                            for j, (si, wi) in enumerate(sl):
                                nc.tensor.matmul(
                                    out=adg[:, t:t + 1],
                                    lhsT=ohtv[:, si, :],
                                    rhs=adwin2[:, wi, :],
                                    start=(j == 0), stop=(j == len(sl) - 1))

                        e_t = sb.tile([P, NTMAX], FP, tag="e_t2", bufs=2)
                        nc.vector.tensor_tensor(
                            out=e_t[:, 0:nt, None],
                            in0=g2v[:, 0:nt, C2 + 1:C2 + 2],
                            in1=adg[:, 0:nt, None], op=OP.add)
                        ee = sb.tile([P, NTMAX], FP, tag="ee2", bufs=2)
                        nc.vector.scalar_tensor_tensor(
                            out=ee[:, 0:nt], in0=e_t[:, 0:nt], scalar=0.2,
                            in1=e_t[:, 0:nt], op0=OP.mult, op1=OP.max)
                        eex = sb.tile([P, NTMAX], BF, tag="eex2", bufs=2)
                        nc.scalar.activation(out=eex[:, 0:nt],
                                             in_=ee[:, 0:nt], func=AF.Exp)

                        msg2 = sb.tile([P, NTMAX * (C2 + 1)], BF, tag="msg2",
                                       bufs=2)
                        msg2v = msg2[:].rearrange("p (t e) -> p t e",
                                                  e=C2 + 1)
                        nc.vector.tensor_tensor(
                            out=msg2v[:, 0:nt, :],
                            in0=g2v[:, 0:nt, 0:C2 + 1],
                            in1=eex[:, 0:nt, None].to_broadcast(
                                [P, nt, C2 + 1]),
                            op=OP.mult)

                        for wi in g:
                            aggp = ps.tile([P, 33], FP, tag="small",
                                           space="PSUM", bufs=2)
                            sl = agg[wi]
                            for j, si in enumerate(sl):
                                t = _slot_tile(slot_list, si)
                                nc.tensor.matmul(
                                    out=aggp[:, 0:C2 + 1],
                                    lhsT=ohgv[:, si, :],
                                    rhs=msg2v[:, t, :],
                                    start=(j == 0), stop=(j == len(sl) - 1))
                            if sweep == 0:
                                nc.scalar.copy(out=acc2[:, wi, :],
                                               in_=aggp[:, 0:C2 + 1])
                            else:
                                self_tail2(nc, cfg, sb, acc2, aggp, wi,
                                           b2sb, ybuf, sxp)

                # final: batched log-softmax tail + one output DMA
                lse = sb.tile([P, WPC], FP, tag="lse")
                nc.scalar.activation(out=lse[:], in_=sxp[:], func=AF.Ln)
                nc.vector.tensor_tensor(
                    out=ybuf[:], in0=ybuf[:],
                    in1=lse[:, :, None].to_broadcast([P, WPC, C2]),
                    op=OP.subtract)
                nc.sync.dma_start(
                    out=out_d[:].rearrange("(w p) c -> p w c", p=P),
                    in_=ybuf[:])

    nc.compile()
    return nc


def _slots(sg):
    """Recompute slot list + per-window slot map from a segment record."""
    g, nt, offs = sg["g"], sg["nt"], sg["offs"]
    slot_list = []
    agg = {wi: [] for wi in g}
    si = 0
    for t in range(nt):
        lo, hi = t * P, (t + 1) * P
        for k, wi in enumerate(g):
            if offs[k] < hi and offs[k + 1] > lo:
                slot_list.append((si, t, wi))
                agg[wi].append(si)
                si += 1
    return slot_list, agg


def _slot_tile(slot_list, si):
    for (s, t, w) in slot_list:
        if s == si:
            return t
    raise KeyError(si)


def self_tail1(nc, cfg, sb, ps, acc1, aggp, wi, Rb, ident_b, b1sb, W2sb,
               t2l, adwin2):
    """Window tail, layer 1: combine halves, normalize, un-rotate, +b1,
    ELU, layer-2 projections -> t2 table row."""
    H, C1, C2, ROW1 = cfg.H, cfg.C1, cfg.C2, cfg.ROW1
    full = sb.tile([P, ROW1 + H], FP, tag="full", bufs=2)
    nc.vector.tensor_tensor(out=full[:], in0=acc1[:, wi, :], in1=aggp[:],
                            op=OP.add)
    den = sb.tile([P, H], FP, tag="den", bufs=2)
    nc.vector.tensor_scalar(out=den[:], in0=full[:, ROW1:ROW1 + H],
                            scalar1=1e-16, scalar2=None, op0=OP.add)
    rden = sb.tile([P, H], FP, tag="rden", bufs=2)
    nc.vector.reciprocal(out=rden[:], in_=den[:])
    normu = sb.tile([P, ROW1], BF, tag="normu", bufs=2)
    nc.vector.tensor_tensor(
        out=normu[:].rearrange("p (h c) -> p h c", c=C1),
        in0=full[:, 0:ROW1].rearrange("p (h c) -> p h c", c=C1),
        in1=rden[:, :, None].to_broadcast([P, H, C1]), op=OP.mult)
    unT = sb.tile([P, 2, P], BF, tag="unT", bufs=2)
    for ch in range(2):
        pt = ps.tile([P, P], BF, tag="tposeb", space="PSUM", bufs=2)
        nc.tensor.transpose(out=pt[:], in_=normu[:, ch * P:(ch + 1) * P],
                            identity=ident_b[:])
        nc.scalar.copy(out=unT[:, ch, :], in_=pt[:])
    hps2 = ps.tile([P, ROW1], FP, tag="big", space="PSUM", bufs=2)
    for ch in range(2):
        nc.tensor.matmul(out=hps2[:, ch * P:(ch + 1) * P],
                         lhsT=unT[:, ch, :], rhs=Rb[:, ch, :],
                         start=True, stop=True)
    xb = sb.tile([P, ROW1], FP, tag="xb", bufs=2)
    nc.vector.tensor_tensor(out=xb[:], in0=hps2[:], in1=b1sb[:], op=OP.add)
    mn = sb.tile([P, ROW1], FP, tag="mn", bufs=2)
    nc.vector.tensor_scalar(out=mn[:], in0=xb[:], scalar1=0.0, scalar2=None,
                            op0=OP.min)
    ex2 = sb.tile([P, ROW1], FP, tag="ex2", bufs=2)
    nc.scalar.activation(out=ex2[:], in_=mn[:], func=AF.Exp)
    z0 = sb.tile([P, ROW1], FP, tag="z0", bufs=2)
    nc.vector.tensor_scalar(out=z0[:], in0=xb[:], scalar1=0.0, scalar2=-1.0,
                            op0=OP.max, op1=OP.add)
    z1 = sb.tile([P, ROW1], BF, tag="z1", bufs=2)
    nc.vector.tensor_tensor(out=z1[:], in0=z0[:], in1=ex2[:], op=OP.add)
    # layer-2 projections
    z1T = sb.tile([P, 2, P], BF, tag="z1T", bufs=2)
    for ch in range(2):
        pt = ps.tile([P, P], BF, tag="tposeb", space="PSUM", bufs=2)
        nc.tensor.transpose(out=pt[:], in_=z1[:, ch * P:(ch + 1) * P],
                            identity=ident_b[:])
        nc.scalar.copy(out=z1T[:, ch, :], in_=pt[:])
    h2ps = ps.tile([P, 34], FP, tag="small", space="PSUM", bufs=2)
    for ch in range(2):
        nc.tensor.matmul(out=h2ps[:], lhsT=z1T[:, ch, :],
                         rhs=W2sb[:, ch, :], start=(ch == 0), stop=(ch == 1))
    t2 = sb.tile([P, cfg.ROW2], BF, tag="t2", bufs=2)
    nc.gpsimd.memset(t2[:, C2 + 2:], 0.0)
    nc.gpsimd.memset(t2[:, C2:C2 + 1], 1.0)
    nc.scalar.copy(out=t2[:, 0:C2], in_=h2ps[:, 0:C2])
    nc.scalar.copy(out=t2[:, C2 + 1:C2 + 2], in_=h2ps[:, C2:C2 + 1])
    nc.scalar.copy(out=adwin2[:, wi, :], in_=h2ps[:, C2 + 1:C2 + 2])
    h = 0 if wi < cfg.H0W else 1
    k = wi if h == 0 else wi - cfg.H0W
    nc.sync.dma_start(out=t2l[h][k * P:(k + 1) * P, :], in_=t2[:])


def self_tail2(nc, cfg, sb, acc2, aggp, wi, b2sb, ybuf, sxp):
    """Window tail, layer 2: combine halves, normalize, +b2, partial
    log-softmax (store shifted values + exp-sum)."""
    C2 = cfg.C2
    full = sb.tile([P, C2 + 1], FP, tag="full2", bufs=2)
    nc.vector.tensor_tensor(out=full[:], in0=acc2[:, wi, :],
                            in1=aggp[:, 0:C2 + 1], op=OP.add)
    den = sb.tile([P, 1], FP, tag="den2", bufs=2)
    nc.vector.tensor_scalar(out=den[:], in0=full[:, C2:C2 + 1],
                            scalar1=1e-16, scalar2=None, op0=OP.add)
    rden = sb.tile([P, 1], FP, tag="rden2", bufs=2)
    nc.vector.reciprocal(out=rden[:], in_=den[:])
    y = sb.tile([P, C2], FP, tag="y", bufs=2)
    nc.vector.scalar_tensor_tensor(out=y[:], in0=full[:, 0:C2],
                                   scalar=rden[:, 0:1], in1=b2sb[:],
                                   op0=OP.mult, op1=OP.add)
    mx = sb.tile([P, 1], FP, tag="mx", bufs=2)
    nc.vector.tensor_reduce(out=mx[:], in_=y[:], op=OP.max,
                            axis=mybir.AxisListType.X)
    nc.vector.tensor_scalar(out=ybuf[:, wi, :], in0=y[:],
                            scalar1=mx[:, 0:1], scalar2=None,
                            op0=OP.subtract)
    exy = sb.tile([P, C2], FP, tag="exy", bufs=2)
    nc.scalar.activation(out=exy[:], in_=ybuf[:, wi, :], func=AF.Exp,
                         accum_out=sxp[:, wi:wi + 1])


def host_inputs(cfg, inputs, per_core):
    x = np.asarray(inputs["x"], np.float32)
    W1 = np.asarray(inputs["W1"], np.float32)
    a_s1 = np.asarray(inputs["att_src1"], np.float32)
    a_d1 = np.asarray(inputs["att_dst1"], np.float32)
    b1 = np.asarray(inputs["b1"], np.float32)
    W2 = np.asarray(inputs["W2"], np.float32)
    a_s2 = np.asarray(inputs["att_src2"], np.float32)
    a_d2 = np.asarray(inputs["att_dst2"], np.float32)
    b2 = np.asarray(inputs["b2"], np.float32)
    H, C1 = cfg.H, cfg.C1

    BD = np.zeros((H * C1, H * C1), np.float64)
    Rblk = np.zeros((2 * P, P), np.float64)
    for h in range(H):
        a = a_s1[h].astype(np.float64)
        Q, _ = np.linalg.qr(a[:, None], mode="complete")
        M = np.vstack([a[None, :], Q[:, 1:].T])
        Minv = np.linalg.inv(M)
        BD[h * C1:(h + 1) * C1, h * C1:(h + 1) * C1] = M.T
        r0 = h * C1
        Rblk[r0:r0 + C1, (r0 % P):(r0 % P) + C1] = Minv.T
    W1R = (W1.astype(np.float64) @ BD).astype(np.float32)
    WA1 = np.einsum("fhc,hc->fh", W1.reshape(cfg.F, H, C1),
                    a_d1).astype(np.float32)
    W1cat = np.concatenate([W1R, WA1], axis=1)          # [256, 264]
    W2cat = np.concatenate([W2, (W2 @ a_s2[0])[:, None],
                            (W2 @ a_d2[0])[:, None]], axis=1)  # [256, 34]
    b1rep = np.tile(b1[None, :], (P, 1)).astype(np.float32)
    b2rep = np.tile(b2[None, :], (P, 1)).astype(np.float32)

    in_maps = []
    for c in range(cfg.NC):
        xs = np.zeros((cfg.SLABP, cfg.F), np.float32)
        xs[0:cfg.SLAB] = x[c * cfg.SLAB:(c + 1) * cfg.SLAB]
        in_maps.append({
            "xT": np.ascontiguousarray(xs.T).astype(BF_NP),
            "W1cat": W1cat.astype(BF_NP),
            "Rblk": Rblk.astype(BF_NP),
            "W2cat": W2cat.astype(BF_NP),
            "b1rep": b1rep, "b2rep": b2rep,
            "srcw": per_core[c]["srcw"],
            "oht": per_core[c]["oht"],
            "ohg": per_core[c]["ohg"],
        })
    return in_maps


_CACHE = {}


def prepare(inputs, cfg=None, reps=1, abl=(), nq=4, subnt=4, split=1.0,
            sort_src=False, sp1=False):
    if cfg is None:
        cfg = GATConfig(n_nodes=inputs["x"].shape[0],
                        n_edges=inputs["edge_index"].shape[1])
    if split is not None:
        cfg.SPLIT = split
    key = (cfg.N, cfg.E, cfg.NC, cfg.GRP, cfg.SPLIT, reps, tuple(abl), nq,
           subnt, sort_src, sp1,
           hash(np.asarray(inputs["edge_index"]).tobytes()))
    if key not in _CACHE:
        segs, TT, TS, per_core = preprocess(cfg, inputs["edge_index"],
                                            sort_src=sort_src)
        nc = build_program(cfg, segs, TT, TS, reps=reps, abl=abl, nq=nq,
                           subnt=subnt, sp1=sp1)
        _CACHE[key] = (cfg, nc, per_core)
    cfg, nc, per_core = _CACHE[key]
    in_maps = host_inputs(cfg, inputs, per_core)
    return cfg, nc, in_maps


def kernel(**inputs):
    cfg, nc, in_maps = prepare(inputs)
    res = run_bass_kernel_spmd(nc, in_maps, core_ids=list(range(cfg.NC)))
    out = np.concatenate(
        [res.results[c]["out"][0:cfg.SLAB] for c in range(cfg.NC)], axis=0)
    return out.astype(np.float32)


def make_runner(cfg, nc, in_maps):
    """Persistent jitted callable with device-resident inputs for timing."""
    import jax
    from jax.sharding import Mesh, PartitionSpec
    from jax.experimental.shard_map import shard_map
    from concourse import bass2jax, mybir as mb

    bass2jax.install_neuronx_cc_hook()
    n_cores = cfg.NC
    partition_name = (nc.partition_id_tensor.name
                      if nc.partition_id_tensor else None)
    in_names, out_names, out_avals, zero_outs = [], [], [], []
    for alloc in nc.m.functions[0].allocations:
        if not isinstance(alloc, mb.MemoryLocationSet):
            continue
        name = alloc.memorylocations[0].name
        if alloc.kind == "ExternalInput":
            if name != partition_name:
                in_names.append(name)
        elif alloc.kind == "ExternalOutput":
            shape = tuple(alloc.tensor_shape)
            dtype = mb.dt.np(alloc.dtype)
            out_names.append(name)
            out_avals.append(jax.core.ShapedArray(shape, dtype))
            zero_outs.append(np.zeros(shape, dtype))
    n_params = len(in_names)
    all_in = list(in_names) + list(out_names)
    if partition_name is not None:
        all_in.append(partition_name)

    def _body(*args):
        operands = list(args)
        if partition_name is not None:
            operands.append(bass2jax.partition_id_tensor())
        outs = bass2jax._bass_exec_p.bind(
            *operands, out_avals=tuple(out_avals), in_names=tuple(all_in),
            out_names=tuple(out_names), lowering_input_output_aliases=(),
            sim_require_finite=False, sim_require_nnan=False, nc=nc)
        return tuple(outs)

    devices = jax.devices()[:n_cores]
    mesh = Mesh(np.asarray(devices), ("core",))
    in_specs = (PartitionSpec("core"),) * (n_params + len(out_names))
    out_specs = (PartitionSpec("core"),) * len(out_names)
    sharded = jax.jit(shard_map(_body, mesh=mesh, in_specs=in_specs,
                                out_specs=out_specs, check_rep=False),
                      keep_unused=True)
    concat_in = [np.concatenate([np.asarray(in_maps[c][nm])
                                 for c in range(n_cores)], axis=0)
                 for nm in in_names]
    dev_in = [jax.device_put(a) for a in concat_in]
    concat_zeros = [
        jax.device_put(np.zeros((n_cores * z.shape[0], *z.shape[1:]),
                                z.dtype))
        for z in zero_outs]

    def run():
        outs = sharded(*dev_in, *concat_zeros)
        jax.block_until_ready(outs)
        return outs

    return run, out_names, out_avals

